# revision 1
# baseline (speedup 1.0000x reference)
"""Trainium2 Bass kernel: soft-top-k-masked pseudo-diagonal fully connected layer.

Computes, for x [16, 1024, 768], V [2304, 768], alpha [2304]:
    m  = dykstra_capped_simplex(alpha / 0.01, k=231, 50 iters)        # [2304]
    W[o, j] = m[(o - j) % 2304] * V[(o - j) % 2304, j]                # [2304, 768]
    out = x @ W.T                                                     # [16, 1024, 2304]

Key identities / structure:
  * Dykstra on the capped simplex reduces to a scalar recursion on w:
        w_1     = z + (k - sum(z)) / n
        w_{i+1} = w_i + (k - sum(clip(w_i, 0, 1))) / n     (49 times)
        m       = clip(w_50, 0, 1)
    Implemented on a [32, 72] layout: DVE clip+accum -> 32 partials,
    [32,32] ones-matmul on the PE sums+broadcasts them, DVE applies the
    per-partition scalar update.  The whole 50-iteration chain is the
    serial prefix of the kernel (the GEMM needs masked weights).
  * The scatter .at[rows, cols].add is a bijection per column, so
        W.T[j, o] = m[(o - j) % 2304] * V.T[j, (o - j) % 2304]
    i.e. row j of W.T is row j of V.T cyclically shifted right by j, scaled
    by a skewed broadcast of m.  The HOST pre-builds ext[768, 2944] fp16 with
    ext[j, 128 + c] = V.T[j, c mod 2304] for c in [-128, 2816) (pure layout
    marshaling, no math).  Weights are held COLUMN-ROTATED per j-block:
        wt[b][dj, u] = W.T[j0+dj, (j0 + u) mod 2304],  u in [0, 2816)
    so each partition row is ONE contiguous DRAM read (ext row j0+dj,
    columns [128 - dj, 128 - dj + 2816)): 128 descriptors per j-block, and
    DMA-ring descriptor feed (~32 ns/descriptor, the real DMA bottleneck)
    stays cheap.  The 512-column extension means every 512-wide GEMM
    o-chunk [o0, o1) maps to the unwrapped u-range [(o0 - j0) mod 2304, +cw).
  * In rotated coordinates the mask is b-INDEPENDENT:
        mask for wt[b][dj, u] = m[(u - dj) mod 2304] = msk[dj, u]
    one [128, 2816] tile read from a 130x-replicated m_rep DRAM buffer with
    partition stride 2303 (== -1 mod 2304); applied to each wt[b] with one
    DVE multiply.
  * x is host-pre-transposed and block-packed to xt[dj, b*2048 + t] fp16
    (one DMA, 128 x 24KB descriptors).  The 16 x 5 x 6 fp16 matmul grid
    (PSUM f32 accumulate over the 6 j-blocks) starts as soon as the mask
    lands.  Output is stored fp16 (upcast on host); fp16 end-to-end keeps
    rel err ~5e-4 << 2e-2 tolerance.

Sharding: data-parallel over the 16384 tokens -> 2048 tokens per core on 8
cores; V/alpha replicated (per the sharding hint). No collectives.
"""

import numpy as np

from concourse import bass, bacc, mybir, tile
from concourse import bass_utils
from concourse.ap import AP

F32 = mybir.dt.float32
F32R = mybir.dt.float32r
F16 = mybir.dt.float16

N_CORES = 8
T_FULL = 16 * 1024          # total tokens
T = T_FULL // N_CORES       # tokens per core = 2048
D = 768                     # in features (contraction)
O = 2304                    # out features
P = 2304                    # total perm (mask length)
PAD = 128                   # ext left wrap pad (covers the intra-tile skew)
WTW = P + 512               # rotated wt width: covers unwrapped 512-chunks
EXTW = PAD + WTW            # 2944
KTOP = 231                  # top-k target
NUM_ITER = 50
INV_LR = 100.0              # 1 / 0.01
K_OVER_N = np.float64(KTOP) / np.float64(P)

NT = T // 128               # 16 token tiles per core
NJ = D // 128               # 6 contraction tiles
# Dykstra layout: z as [DYK_P, DYK_F], flat index = q * DYK_F + r
DYK_P = 32
DYK_F = P // DYK_P          # 72
# o-chunks for the main matmul (one PSUM bank each)
O_CHUNKS = [(0, 512), (512, 1024), (1024, 1536), (1536, 2048), (2048, 2304)]


def build_program():
    nc = bacc.Bacc("TRN2", target_bir_lowering=False, debug=False,
                   num_devices=N_CORES)

    xt_d = nc.dram_tensor("xt", [128, NJ * T], F16, kind="ExternalInput")
    ext_d = nc.dram_tensor("ext", [D, EXTW], F16, kind="ExternalInput")
    al_d = nc.dram_tensor("al", [DYK_P, DYK_F], F32, kind="ExternalInput")
    out_d = nc.dram_tensor("out", [T, O], F16, kind="ExternalOutput")
    mrep_d = nc.dram_tensor("m_rep", [130 * P], F16, kind="Internal")

    out_r = out_d.ap().rearrange("(n p) o -> n p o", p=128)  # [16, 128, 2304]

    with tile.TileContext(nc) as tc:
        with (
            tc.tile_pool(name="small", bufs=1) as small,
            tc.tile_pool(name="xtp", bufs=1) as xtp,
            tc.tile_pool(name="wtp", bufs=NJ) as wtp,
            tc.tile_pool(name="mskp", bufs=1) as mskp,
            tc.tile_pool(name="orow", bufs=2) as orow,
            tc.tile_pool(name="ps8", bufs=7, space="PSUM") as ps8,
            tc.tile_pool(name="dk", bufs=1, space="PSUM") as dkp,
        ):
            # ---- alpha first (it heads the Dykstra critical path) ----
            al_t = small.tile([DYK_P, DYK_F], F32, tag="al")
            nc.sync.dma_start(al_t[:], al_d.ap())

            # ---- x / raw-W.T loads (independent of the mask) ----
            xt = xtp.tile([128, NJ * T], F16, tag="xt")
            nc.sync.dma_start(xt[:], xt_d.ap())

            wt = [wtp.tile([128, WTW], F16, tag="wtp", name=f"wt{b}")
                  for b in range(NJ)]
            for b in range(NJ):
                j0 = 128 * b
                # wt[b][dj, u] = ext[j0+dj, PAD - dj + u]: one contiguous
                # span per partition row.
                sk = AP(ext_d, j0 * EXTW + PAD, [[EXTW - 1, 128], [1, WTW]])
                eng = nc.sync if b < 3 else nc.scalar
                eng.dma_start(wt[b][:], sk)

            # ---- Dykstra (the serial critical path) ----
            zeros_t = small.tile([DYK_P, DYK_F], F32, tag="zeros")
            nc.vector.memset(zeros_t[:], 0.0)
            ones_q = small.tile([DYK_P, DYK_P], F32, tag="onesq")
            nc.vector.memset(ones_q[:], 1.0 / float(P))
            w = small.tile([DYK_P, DYK_F], F32, tag="w")
            ctmp = small.tile([DYK_P, DYK_F], F32, tag="ctmp")
            red = small.tile([DYK_P, 1], F32, tag="red")
            m16 = small.tile([DYK_P, DYK_F], F16, tag="m16")

            # w = 100*alpha ; red = per-partition partials of sum(w)
            nc.vector.tensor_scalar(w[:], al_t[:], INV_LR, 0.0,
                                    op0=mybir.AluOpType.mult,
                                    op1=mybir.AluOpType.add,
                                    accum_out=red[:])
            for i in range(NUM_ITER):
                # s = sum(red) / n, broadcast to DYK_P partitions
                s_ps = dkp.tile([DYK_P, 1], F32, tag="dk")
                nc.tensor.matmul(s_ps[:], ones_q[:], red[:],
                                 start=True, stop=True)
                # w += k/n - s
                nc.vector.tensor_scalar(w[:], w[:], s_ps[:], K_OVER_N,
                                        op0=mybir.AluOpType.subtract,
                                        op1=mybir.AluOpType.add)
                if i < NUM_ITER - 1:
                    # ctmp = clip(w, 0, 1); red = partials of sum(ctmp)
                    nc.vector.scalar_tensor_tensor(
                        ctmp[:], w[:], 1.0, zeros_t[:],
                        op0=mybir.AluOpType.min,
                        op1=mybir.AluOpType.max,
                        accum_out=red[:])
            # m (fp16) = clip(w_50, 0, 1)
            nc.vector.scalar_tensor_tensor(m16[:], w[:], 1.0, zeros_t[:],
                                           op0=mybir.AluOpType.min,
                                           op1=mybir.AluOpType.max)

            # ---- m -> m_rep (130x replicated in DRAM for the skewed read)
            mw0 = nc.gpsimd.dma_start(
                AP(mrep_d, 0, [[DYK_F, DYK_P], [1, DYK_F]]), m16[:])
            rep_engines = [nc.sync, nc.scalar, nc.gpsimd]
            rep_writes = [mw0]
            start = 1
            for ei, eng in enumerate(rep_engines):
                ncopy = 43
                mw = eng.dma_start(
                    AP(mrep_d, P * start, [[P, ncopy], [1, P]]),
                    AP(mrep_d, 0, [[0, ncopy], [1, P]]))
                tile.add_dep_helper(mw.ins, mw0.ins, reason="m_rep RAW")
                rep_writes.append(mw)
                start += ncopy
            assert start == 130

            # ---- skewed m broadcast (b-independent) + mask apply ----
            # msk[dj, u] = m_rep[dj*2303 + u] = m[(u - dj) % P]
            msk = mskp.tile([128, WTW], F16, tag="mskp")
            mr = nc.gpsimd.dma_start(
                msk[:], AP(mrep_d, 0, [[P - 1, 128], [1, WTW]]))
            for mw in rep_writes:
                tile.add_dep_helper(mr.ins, mw.ins, reason="m_rep RAW")
            for b in range(NJ):
                nc.vector.tensor_tensor(wt[b][:], wt[b][:], msk[:],
                                        op=mybir.AluOpType.mult)

            # ---- main matmul: out[t, o] = sum_j x[t, j] * W.T[j, o] ----
            for tt in range(NT):
                row = orow.tile([128, O], F16, tag="orow")
                for ci, (o0, o1) in enumerate(O_CHUNKS):
                    ps = ps8.tile([128, 512], F32, tag="ps8")
                    cw = o1 - o0
                    for b in range(NJ):
                        u0 = (o0 - 128 * b) % P
                        nc.tensor.matmul(
                            ps[:, 0:cw],
                            xt[:, T * b + 128 * tt: T * b + 128 * (tt + 1)],
                            wt[b][:, u0:u0 + cw],
                            start=(b == 0), stop=(b == NJ - 1),
                        )
                    if ci % 2 == 0:
                        nc.vector.tensor_copy(row[:, o0:o1], ps[:, 0:cw])
                    else:
                        nc.scalar.copy(row[:, o0:o1], ps[:, 0:cw])
                eng = nc.scalar if tt % 2 == 0 else nc.sync
                eng.dma_start(out_r[tt], row[:])

    nc.compile()
    return nc


_CACHE = {}


def _get_program():
    if "nc" not in _CACHE:
        _CACHE["nc"] = build_program()
    return _CACHE["nc"]


def make_in_maps(x, V, alpha):
    """Host-side layout marshaling: transpose/cast only, no arithmetic."""
    xf = np.asarray(x, dtype=np.float32).reshape(T_FULL, D)
    VT = np.asarray(V, dtype=np.float32).T            # [768, 2304]
    ext = np.concatenate(
        [VT[:, P - PAD:], VT, VT[:, :WTW - P]], axis=1).astype(np.float16)
    al = np.ascontiguousarray(
        np.asarray(alpha, dtype=np.float32).reshape(DYK_P, DYK_F))
    in_maps = []
    for c in range(N_CORES):
        xs = xf[T * c:T * (c + 1)].T.astype(np.float16)   # [768, 2048]
        # pack to xt[dj, b*T + t] = x[t, 128b + dj]
        xt = np.ascontiguousarray(
            xs.reshape(NJ, 128, T).transpose(1, 0, 2).reshape(128, NJ * T))
        in_maps.append({"xt": xt, "ext": ext, "al": al})
    return in_maps


def gather_out(res):
    out = np.concatenate(
        [res.results[c]["out"].astype(np.float32) for c in range(N_CORES)],
        axis=0)
    return out.reshape(16, 1024, O)


def kernel(x, V, alpha):
    nc = _get_program()
    in_maps = make_in_maps(x, V, alpha)
    res = bass_utils.run_bass_kernel_spmd(nc, in_maps,
                                          core_ids=list(range(N_CORES)))
    return gather_out(res)

